# revision 4
# baseline (speedup 1.0000x reference)
"""Distributed Trainium2 (8 NeuronCore) multi-head attention kernel.

Problem: y = softmax((x Wq)(x Wk)^T * DIM**-0.5) (x Wv) Wo + bo
  x: [4096, 256], 8 heads of dim 32, scale by full-dim**-0.5 (1/16).

Sharding: head-parallel. Each core owns one head h.

v2 changes vs the 222us baseline:
  - Scores use 4-way PE row tiling: q^T/k^T are produced REPLICATED x4
    across the partition groups (host replicates the projection weight
    columns, so the QKV matmul emits [128, n] with 4 copies for free).
    Each score group runs 4 concurrent K=32 matmuls on row tiles
    T0/T4/T8/T12 (tile_position auto-derived from base_partition),
    writing 4 different PSUM banks. ~4x fewer PE-walltime per j-tile.
  - PE phases are segregated per pass (scores burst in 32x128 tile
    mode, AV burst in 128x128 mode) at super-slot granularity so the
    tiling-mode-switch drain is paid only 8x per pass.
  - exp split rebalanced: ScalarE 19/32 j-tiles (table Exp), VectorE
    13/32 (Schraudolph int16 bit trick) writing straight into the bf16
    P^T tile through a bitcast AP (no same-engine fixup copy).
  - osl split per half so the final projection of half 0 overlaps the
    second AllToAll.
"""

import numpy as np

P = 128          # partitions
N = 4096         # sequence length
DIM = 256        # model dim
H = 8            # heads == cores
D = DIM // H     # head dim, 32
KC = DIM // P    # 2 contraction chunks
NT = N // P      # 32 j-tiles
NCORES = 8
RPC = N // NCORES   # 512 output rows per core
QW = 256         # q-column width per pass
NPASS = 2 * NCORES  # 16
SCALE = DIM ** -0.5

# Schraudolph bf16 fast-exp: bits(exp(s*SCALE)) ~= s*FE_A + FE_B (int16)
FE_A = 128.0 * SCALE * 1.4426950408889634
FE_B = 16256.0 - 4.6

# Per-pass evac schedule: 8 units (one per score group of 4 j-tiles).
# True = VectorE (Schraudolph), False = ScalarE (table Exp). Unit 7 is
# split: banks 0-2 ScalarE, bank 3 VectorE -> 19 S / 13 D j-tiles.
UNIT_DVE = [False, True, False, True, False, True, False, None]

# j -> (engine, slot-in-engine-tile) maps
J_ENG = {}
_s = _d = 0
for _g in range(8):
    for _i in range(4):
        _j = 4 * _g + _i
        dve = UNIT_DVE[_g] if UNIT_DVE[_g] is not None else (_i == 3)
        if dve:
            J_ENG[_j] = (True, _d)
            _d += 1
        else:
            J_ENG[_j] = (False, _s)
            _s += 1
NSJ, NDJ = _s, _d   # 19, 13

_BUILT = None


def _build():
    from contextlib import ExitStack

    import concourse.mybir as mybir
    import concourse.tile as tile
    from concourse import bacc
    from concourse.masks import make_identity

    f32 = mybir.dt.float32
    bf16 = mybir.dt.bfloat16
    i16 = mybir.dt.int16
    AF = mybir.ActivationFunctionType
    ALU = mybir.AluOpType

    nc = bacc.Bacc("TRN2", target_bir_lowering=False, debug=False,
                   num_devices=NCORES)
    xT = nc.dram_tensor("xT", [DIM, N], bf16, kind="ExternalInput")
    wqr = nc.dram_tensor("wqr", [DIM, P], f32, kind="ExternalInput")
    wkr = nc.dram_tensor("wkr", [DIM, P], f32, kind="ExternalInput")
    wv = nc.dram_tensor("wv", [DIM, D], f32, kind="ExternalInput")
    bqr = nc.dram_tensor("bqr", [P, 1], f32, kind="ExternalInput")
    bkr = nc.dram_tensor("bkr", [P, 1], f32, kind="ExternalInput")
    bv = nc.dram_tensor("bv", [D, 1], f32, kind="ExternalInput")
    wout = nc.dram_tensor("wout", [DIM, DIM], f32, kind="ExternalInput")
    bout = nc.dram_tensor("bout", [1, DIM], f32, kind="ExternalInput")
    out = nc.dram_tensor("out", [RPC, DIM], f32, kind="ExternalOutput")

    with tile.TileContext(nc) as tc, ExitStack() as ctx:
        singles = ctx.enter_context(tc.tile_pool(name="singles", bufs=1))
        sm_pool = ctx.enter_context(tc.tile_pool(name="sm", bufs=3))
        pt_pool = ctx.enter_context(tc.tile_pool(name="ptp", bufs=2))
        # PSUM budget (8 banks): st_all 4 banks + acc_pool 2 + qkv ring 2
        ps_sing = ctx.enter_context(
            tc.tile_pool(name="pss", bufs=1, space="PSUM"))
        qk_pool = ctx.enter_context(
            tc.tile_pool(name="qkp", bufs=2, space="PSUM"))
        acc_pool = ctx.enter_context(
            tc.tile_pool(name="accp", bufs=2, space="PSUM"))
        dram = ctx.enter_context(
            tc.tile_pool(name="dram", bufs=1, space="DRAM"))

        # ---------------- constant / input loads ----------------
        xbf = singles.tile([P, KC, N], bf16)
        for c in range(KC):
            for q4 in range(4):
                sl = slice(q4 * (N // 4), (q4 + 1) * (N // 4))
                nc.sync.dma_start(out=xbf[:, c, sl],
                                  in_=xT[c * P:(c + 1) * P, sl])

        def _ldw(t, cols):
            w32 = singles.tile([P, KC, cols], f32, name=f"w32{t.name}",
                               tag=f"w32{t.name}")
            for c in range(KC):
                nc.sync.dma_start(out=w32[:, c, :], in_=t[c * P:(c + 1) * P, :])
            wbf = singles.tile([P, KC, cols], bf16, name=f"wbf{t.name}",
                               tag=f"wbf{t.name}")
            nc.vector.tensor_copy(wbf[:], w32[:])
            return wbf

        wqbf = _ldw(wqr, P)
        wkbf = _ldw(wkr, P)
        wvbf = _ldw(wv, D)
        wobf = _ldw(wout, DIM)

        bq_t = singles.tile([P, 1], f32, name="bqt", tag="bqt")
        nc.sync.dma_start(out=bq_t[:], in_=bqr[:, :])
        bk_t = singles.tile([P, 1], f32, name="bkt", tag="bkt")
        nc.sync.dma_start(out=bk_t[:], in_=bkr[:, :])
        bv_t = singles.tile([D, 1], f32, name="bvt", tag="bvt")
        nc.sync.dma_start(out=bv_t[:], in_=bv[:, :])

        bo32 = singles.tile([1, DIM], f32)
        nc.sync.dma_start(out=bo32[:], in_=bout[:, :])
        bobf = singles.tile([1, DIM], bf16)
        nc.vector.tensor_copy(bobf[:], bo32[:])

        ones1 = singles.tile([1, P], bf16)
        nc.vector.memset(ones1[:], 1.0)
        ident = singles.tile([P, P], bf16)
        make_identity(nc, ident[:])

        # ------- QKV projection (128x128 tile mode) ----------------
        # qTr/kTr [128, 4096]: partition group i = replica i of q^T/k^T
        # (weights arrive host-replicated). vT [32, 4096] single copy.
        qTr = singles.tile([P, N], bf16)
        kTr = singles.tile([P, N], bf16)
        vT = singles.tile([D, N], bf16)
        FT2 = 512
        emits = [(wqbf, bq_t, qTr, P), (wkbf, bk_t, kTr, P),
                 (wvbf, bv_t, vT, D)]
        for g, (wbf, bt, dst, m) in enumerate(emits):
            for t in range(N // FT2):   # 8
                ps = qk_pool.tile([P, FT2], f32, tag="qk", name=f"qk{g}_{t}")
                sl = slice(t * FT2, (t + 1) * FT2)
                for c in range(KC):
                    nc.tensor.matmul(
                        ps[:m, :], lhsT=wbf[:, c, :],
                        rhs=xbf[:, c, sl],
                        start=(c == 0), stop=(c == KC - 1))
                if (g * 8 + t) % 2 == 0:
                    nc.vector.tensor_scalar_add(dst[:, sl], ps[:m, :], bt[:])
                else:
                    nc.scalar.activation(dst[:, sl], ps[:m, :], AF.Identity,
                                         bias=bt[:, 0:1])

        # ------- v -> [128 j, 32 d] tiles (+ ones col), 32x128 mode -----
        vsb = singles.tile([P, NT, D + 1], bf16)
        vt = qk_pool.tile([P, NT * D], bf16, tag="qk", name="vt")
        for j in range(NT):
            nc.tensor.transpose(vt[:, j * D:(j + 1) * D],
                                vT[:, j * P:(j + 1) * P],
                                ident[:D, :D])
        nc.vector.tensor_copy(vsb[:, :, 0:D],
                              vt.rearrange("p (j d) -> p j d", j=NT))
        nc.vector.memset(vsb[:, :, D:D + 1], 1.0)

        # ---------------- attention main loop ----------------
        # st_all [128, bank(4), slot(2), 256] f32 = 4 PSUM banks.
        # Score group g (j = 4g..4g+3) runs 4 concurrent row-tile matmuls
        # (row tile i reads kTr/qTr partitions [32i,32i+32)) into
        # st_all[:, i, g%2, :]. Evac unit g reads st_all[:, :, g%2, :].
        st_all = ps_sing.tile([P, 4, 2, QW], f32, tag="st", name="st_all")

        otb = singles.tile([P, NCORES, P], bf16)
        a2a_in = [dram.tile([NCORES, D, QW], bf16, name=f"a2ai{h_}",
                            tag=f"a2ai{h_}") for h_ in range(2)]
        a2a_out = [dram.tile([NCORES, D, QW], bf16, name=f"a2ao{h_}",
                             tag=f"a2ao{h_}") for h_ in range(2)]
        osl = [singles.tile([P, KC, QW], bf16, name=f"osl{h_}",
                            tag=f"osl{h_}") for h_ in range(2)]

        def emit_half_a2a(hp):
            for c in range(NCORES):
                pp = hp * NCORES + c
                G = pp // 2
                for x in range(2):
                    k = (pp % 2) * 2 + x
                    nc.sync.dma_start(
                        out=a2a_in[hp][c][:, x * P:(x + 1) * P],
                        in_=otb[k * D:(k + 1) * D, G, :])
            nc.gpsimd.collective_compute(
                "AllToAll", ALU.bypass,
                replica_groups=[list(range(NCORES))],
                ins=[a2a_in[hp][:].opt()], outs=[a2a_out[hp][:].opt()])
            a2a_flat = a2a_out[hp].rearrange("c d r -> (c d) r")
            for c in range(KC):
                nc.sync.dma_start(
                    out=osl[hp][:, c, :],
                    in_=a2a_flat[c * P:(c + 1) * P, :])

        def emit_half_proj(hp):
            for m2 in range(QW // P):
                mt = hp * (QW // P) + m2
                fo = acc_pool.tile([P, DIM], f32, tag="acc")
                for c in range(KC):
                    nc.tensor.matmul(
                        fo[:], lhsT=osl[hp][:, c, m2 * P:(m2 + 1) * P],
                        rhs=wobf[:, c, :], start=(c == 0), stop=False)
                nc.tensor.matmul(fo[:], lhsT=ones1[:], rhs=bobf[:],
                                 start=False, stop=True)
                fout = sm_pool.tile([P, DIM], f32, tag="fout")
                nc.vector.tensor_copy(fout[:], fo[:])
                nc.sync.dma_start(out=out[mt * P:(mt + 1) * P, :],
                                  in_=fout[:])

        # S-engine and D-engine P^T tiles (double-buffered across passes)
        ptqs = [None, None]
        ptqd = [None, None]
        acc = [None, None]
        obq = [None]

        for p in range(NPASS + 1):
            if p < NPASS:
                ptqs[p % 2] = pt_pool.tile([P, NSJ, QW], bf16, tag="pts",
                                           name=f"ptqs{p}")
                ptqd[p % 2] = pt_pool.tile([P, NDJ, QW], bf16, tag="ptd",
                                           name=f"ptqd{p}")
            base = (p % NCORES) * RPC + (p // NCORES) * QW
            for ss in range(4):   # super-slots
                # ---- scores (32x128 row-tiled): groups 2ss, 2ss+1 ----
                if p < NPASS:
                    for g in (2 * ss, 2 * ss + 1):
                        s = g % 2
                        for i in range(4):
                            j = 4 * g + i
                            nc.tensor.matmul(
                                st_all[:, i, s, :],
                                lhsT=kTr[32 * i:32 * (i + 1),
                                         j * P:(j + 1) * P],
                                rhs=qTr[32 * i:32 * (i + 1),
                                        base:base + QW],
                                start=True, stop=True,
                                tile_position=(32 * i, 0))
                        # evac unit g
                        if UNIT_DVE[g] is None:
                            sj0 = J_ENG[4 * g][1]
                            nc.scalar.activation(
                                ptqs[p % 2][:, sj0:sj0 + 3, :],
                                st_all[:, 0:3, s, :], AF.Exp,
                                scale=float(SCALE))
                            dj0 = J_ENG[4 * g + 3][1]
                            nc.vector.tensor_scalar(
                                ptqd[p % 2][:, dj0:dj0 + 1, :].bitcast(i16),
                                st_all[:, 3:4, s, :],
                                scalar1=float(FE_A), scalar2=float(FE_B),
                                op0=ALU.mult, op1=ALU.add)
                        elif UNIT_DVE[g]:
                            dj0 = J_ENG[4 * g][1]
                            nc.vector.tensor_scalar(
                                ptqd[p % 2][:, dj0:dj0 + 4, :].bitcast(i16),
                                st_all[:, :, s, :],
                                scalar1=float(FE_A), scalar2=float(FE_B),
                                op0=ALU.mult, op1=ALU.add)
                        else:
                            sj0 = J_ENG[4 * g][1]
                            nc.scalar.activation(
                                ptqs[p % 2][:, sj0:sj0 + 4, :],
                                st_all[:, :, s, :], AF.Exp,
                                scale=float(SCALE))
                # ---- AV for pass p-1 (128x128 mode): 16 j per ss ----
                if p > 0:
                    pts, ptd = ptqs[(p - 1) % 2], ptqd[(p - 1) % 2]
                    qb2 = ss // 2
                    if ss % 2 == 0:
                        acc[qb2] = acc_pool.tile([P, D + 1], f32, tag="acc",
                                                 name=f"acc{p}_{qb2}")
                    coff = qb2 * P
                    for j in range(16 * (ss % 2), 16 * (ss % 2) + 16):
                        dve, sl_ = J_ENG[j]
                        src = (ptd if dve else pts)[:, sl_, coff:coff + P]
                        nc.tensor.matmul(acc[qb2][:], lhsT=src,
                                         rhs=vsb[:, j, :],
                                         start=(j == 0), stop=(j == NT - 1))
                    if ss % 2 == 1:
                        pp = p - 1
                        k = (pp % 2) * 2 + qb2
                        G = pp // 2
                        if k == 0:
                            obq[0] = sm_pool.tile([P, 4, D], bf16,
                                                  tag="obq", name=f"obq{G}")
                        r = sm_pool.tile([P, 1], f32, tag="r",
                                         name=f"r{p}_{qb2}")
                        nc.vector.reciprocal(r[:], acc[qb2][:, D:D + 1])
                        nc.scalar.activation(obq[0][:, k, :],
                                             acc[qb2][:, 0:D],
                                             AF.Copy, scale=r[:, 0:1])
                        if k == 3:
                            nc.sync.dma_start_transpose(otb[:, G, :],
                                                        obq[0][:])
            if p - 1 == NCORES - 1:
                emit_half_a2a(0)
        emit_half_a2a(1)
        emit_half_proj(0)
        emit_half_proj(1)

    nc.compile()
    return nc


def _get_built():
    global _BUILT
    if _BUILT is None:
        _BUILT = _build()
    return _BUILT


def make_in_maps(x, w_qkv, b_qkv, w_out, b_out):
    x = np.asarray(x, dtype=np.float32)
    w_qkv = np.asarray(w_qkv, dtype=np.float32)
    b_qkv = np.asarray(b_qkv, dtype=np.float32)
    w_out = np.asarray(w_out, dtype=np.float32)
    b_out = np.asarray(b_out, dtype=np.float32)

    import ml_dtypes
    xT = np.ascontiguousarray(x.T).astype(ml_dtypes.bfloat16)
    wq3 = w_qkv.reshape(DIM, 3, H, D)       # [in, (q|k|v), head, d]
    bq3 = b_qkv.reshape(3, H, D)
    in_maps = []
    for h in range(NCORES):
        in_maps.append({
            "xT": xT,
            "wqr": np.ascontiguousarray(np.tile(wq3[:, 0, h, :], (1, 4))),
            "wkr": np.ascontiguousarray(np.tile(wq3[:, 1, h, :], (1, 4))),
            "wv": np.ascontiguousarray(wq3[:, 2, h, :]),
            "bqr": np.ascontiguousarray(np.tile(bq3[0, h], 4)[:, None]),
            "bkr": np.ascontiguousarray(np.tile(bq3[1, h], 4)[:, None]),
            "bv": np.ascontiguousarray(bq3[2, h][:, None]),
            "wout": np.ascontiguousarray(w_out),
            "bout": np.ascontiguousarray(b_out.reshape(1, DIM)),
        })
    return in_maps


def kernel(x, w_qkv, b_qkv, w_out, b_out):
    from concourse.bass_utils import run_bass_kernel_spmd

    nc = _get_built()
    in_maps = make_in_maps(x, w_qkv, b_qkv, w_out, b_out)
    res = run_bass_kernel_spmd(nc, in_maps, core_ids=list(range(NCORES)))
    return np.concatenate([res.results[i]["out"] for i in range(NCORES)],
                          axis=0)
